# revision 48
# baseline (speedup 1.0000x reference)
"""Causal self-attention (B=4, T=2048, D=1024, H=16) on 8 TRN2 NeuronCores.

Sharding: core c handles batch b = c // 2 and head-group g = c % 2
(8 heads = 512 of the 1024 feature dims). Each core:
  1. QKV projection for its head-group's columns. q, k are produced
     TRANSPOSED ([feat, tok], feature dim on partitions) so they feed the
     attention matmuls directly; v is produced natural ([tok, feat]) so it
     is the PV stationary operand.
  2. RoPE via a PE rotation matmul (rotate_half as a constant 128x128
     block-diagonal permutation) + DVE combine with cos/sin. The PSUM
     bias-add/evacuation runs on the ACT engine (Identity + bias AP),
     leaving the cos/sin muls as fast 2-byte DVE ops.
  3. Causal attention with scores in [k, q] orientation: exp(score/8 - 2)
     without max-subtraction (shift-invariant), row-sum obtained free via a
     ones-column appended to v (PV matmul M=65: rows 0-63 = y, row 64 =
     softmax denominator).
  4. Late softmax normalization (reciprocal + gpsimd partition-broadcast),
     then the partial output projection with its 512 rows of W_out.
Host sums the two head-group partials per batch and adds b_out.

Precision plan (gate: rel l2 < 2e-2; measured 1.802e-2):
  - fp16 everywhere bf16 was used (same PE/DVE cost, 8x less rounding).
  - Scores for q-chunks 1-3 run as fp8e4m3 DoubleRow matmuls at HALF the
    cycles-per-column: lhsT packs k as an error-compensated (hi, err) row
    pair, rhs is pure-fp8 q broadcast across the row dim (stride-0 AP), so
    out = q8 . (k8h + k8e) = q8 . k_exact in ONE 0.5-rate matmul. Chunk 0
    keeps fp16 scores: its early tokens have concentrated softmax rows
    (few attendable keys) that amplify quantization straight into y.
  - PV tiles contract an error-compensated fp8 (hi, err) v pair stored
    interleaved: dense tiles as DoubleRow over (j0, j1) k-tile rows (one
    matmul each for hi and err), chunks 1-3's diag tiles as DoubleRow
    over the (hi, err) rows against a row-broadcast fp8 exp. Chunk 0's
    diag PV stays fp16 with fp16 exp (same early-token argument).
    Softmax statistics accumulate in fp32 PSUM.
  - The causal mask is applied pre-exp as -800 added into the scores
    PSUM by an N=128 matmul (identity x mask), so exp writes exact zeros
    and no DVE op sits on the exp->PV chain.

Schedule: token chunks (512 q each) outermost, head pairs inner; each
block's thin diagonal score tiles run first so their longer
scores->exp->PV chains overlap the dense tiles' PE work. Projection /
v / out-proj work items stream into the attention pipeline's PE gaps
under a per-chunk quota that saves the out-projections for the last
(exp-bound) chunk; the last chunk's out-proj splits its accumulation so
only the final head-pair matmuls wait on the last epilogue. DMA layouts
keep per-partition runs >= 512B (sub-512B runs pay 2x in the DMA
engines) and the startup loads alternate the two HWDGE queues in
consumption order.
"""

import numpy as np

import concourse.tile as tile
from concourse import bacc, mybir
from concourse.bass_utils import run_bass_kernel_spmd

dt = mybir.dt
f16 = np.float16

B, T, C = 4, 2048, 1024
H, HD = 16, 64
N_CORES = 8
HPC = 8          # heads per core
KSUB = C // 128  # 8 contraction subtiles
TT = T // 128    # 16 token tiles
TC = T // 512    # 4 token chunks

_compiled = None


def _build():
    nc = bacc.Bacc()
    dts = dt.float16
    dt8 = dt.float8e4

    xT = nc.dram_tensor("xT", [TC, 128, KSUB, 512], dts, kind="ExternalInput")
    Wqk = nc.dram_tensor("Wqk", [8, 128, KSUB, 128], dts, kind="ExternalInput")
    Wv = nc.dram_tensor("Wv", [128, KSUB, 512], dts, kind="ExternalInput")
    Wo = nc.dram_tensor("Wo", [128, 4, 1024], dts, kind="ExternalInput")
    bqk = nc.dram_tensor("bqk", [128, 8], dt.float32, kind="ExternalInput")
    bv = nc.dram_tensor("bv", [128, 512], dt.float32, kind="ExternalInput")
    RT = nc.dram_tensor("RT", [128, 128], dts, kind="ExternalInput")
    cosd = nc.dram_tensor("cosd", [TC, 128, 512], dts, kind="ExternalInput")
    sind = nc.dram_tensor("sind", [TC, 128, 512], dts, kind="ExternalInput")
    maskd = nc.dram_tensor("maskd", [128, 128], dts, kind="ExternalInput")
    identd = nc.dram_tensor("identd", [128, 128], dts, kind="ExternalInput")
    out = nc.dram_tensor("out", [T, C], dts, kind="ExternalOutput")

    with tile.TileContext(nc) as tc:
        with (
            tc.tile_pool(name="weights", bufs=1) as wp,
            tc.tile_pool(name="acts", bufs=1) as ap,
            tc.tile_pool(name="scratch", bufs=2) as sp,
            tc.tile_pool(name="exps", bufs=7) as ep,
            tc.tile_pool(name="norm", bufs=2) as np_,
            tc.tile_pool(name="outs", bufs=3) as op,
            tc.tile_pool(name="psum", bufs=2, space="PSUM") as pp,
            tc.tile_pool(name="psum_big", bufs=2, space="PSUM") as pb,
            tc.tile_pool(name="psum_pv", bufs=2, space="PSUM") as ppv,
        ):
            # chunk-major xT and fs-major Wqk: DMA destination runs are
            # 8KB/2KB contiguous per partition (sub-512B runs pay 2x in the
            # DMA engines)
            xT_sb = wp.tile([128, TC, KSUB, 512], dts)
            Wqk_sb = wp.tile([128, 8, KSUB, 128], dts)
            Wv_sb = wp.tile([128, KSUB, 512], dts)
            Wo_sb = wp.tile([128, 4, 1024], dts)
            bqk_sb = wp.tile([128, 8], dt.float32)
            bv_sb = wp.tile([128, 512], dt.float32)
            RT_sb = wp.tile([128, 128], dts)
            # interleaved [2, T] planes: 0 = sin, 1 = cos, so one wide DVE
            # mul produces both rope products
            sc_sb = wp.tile([128, 2, T], dts)
            # mneg[q, kj] = -800 above the diagonal: added into the scores
            # PSUM via a tiny N=128 matmul (vs ident) so exp() zeroes the
            # masked triangle with no DVE op on the exp->PV chain
            mask_sb = wp.tile([128, 128], dts)
            ident_sb = wp.tile([128, 128], dts)
            # exp bias constant (-2): keeps fp8e4m3 exp outputs under the
            # 448 max (softmax is shift-invariant, numerator and denominator
            # both scale by e^-2)
            nb2 = wp.tile([128, 1], dt.float32)
            nc.vector.memset(nb2[:], -2.0)

            def load_xT(c4):
                nc.sync.dma_start(xT_sb[:, c4], xT[c4])

            def load_wqk(fs):
                nc.sync.dma_start(Wqk_sb[:, fs], Wqk[fs])

            # first-needed data up front, in consumption order, alternating
            # the two HWDGE queues (SP + Activation) so descriptor issue
            # (fixed cost per DMA) pipelines with the transfers
            cosv = sc_sb[:, 1].rearrange("p (c q) -> p c q", c=TC)
            sinv = sc_sb[:, 0].rearrange("p (c q) -> p c q", c=TC)
            # proj(0,0)'s full chain (Wqk0 -> xT0 -> bias -> sin/cos -> RT)
            # loads first so the first rope completes with no DMA stalls
            load_wqk(0)
            nc.scalar.dma_start(xT_sb[:, 0, 0:4], xT[0][:, 0:4, :])
            nc.sync.dma_start(xT_sb[:, 0, 4:8], xT[0][:, 4:8, :])
            nc.scalar.dma_start(bqk_sb[:], bqk[:])
            nc.scalar.dma_start(cosv[:, 0], cosd[0])
            nc.scalar.dma_start(sinv[:, 0], sind[0])
            nc.scalar.dma_start(RT_sb[:], RT[:])
            nc.sync.dma_start(Wqk_sb[:, 4], Wqk[4])
            nc.sync.dma_start(Wv_sb[:], Wv[:])
            nc.scalar.dma_start(bv_sb[:], bv[:])
            nc.scalar.dma_start(mask_sb[:], maskd[:])
            nc.scalar.dma_start(ident_sb[:], identd[:])
            load_wqk(1)
            nc.scalar.dma_start(Wqk_sb[:, 5], Wqk[5])
            load_wqk(2)
            nc.scalar.dma_start(Wqk_sb[:, 6], Wqk[6])
            load_wqk(3)
            nc.scalar.dma_start(Wqk_sb[:, 7], Wqk[7])
            load_xT(1)
            nc.scalar.dma_start(cosv[:, 1], cosd[1])
            nc.scalar.dma_start(sinv[:, 1], sind[1])
            nc.sync.dma_start(Wo_sb[:], Wo[:])
            for c4 in range(2, TC):
                load_xT(c4)
                nc.sync.dma_start(cosv[:, c4], cosd[c4])
                nc.sync.dma_start(sinv[:, c4], sind[c4])

            # chunk-0 q/k stay fp16 (accurate scores for the concentrated
            # early-softmax rows); chunks 1-3 go straight to fp8
            qT_sb = ap.tile([128, 4, 512], dts)     # rope'd q, chunk 0
            kT_sb = ap.tile([128, 4, 512], dts)     # rope'd k, chunk 0
            q8_sb = ap.tile([128, 4, T], dt8)       # rope'd q fp8, chunks 1-3
            # k error-compensated fp8 (hi, err) row pair for DoubleRow
            # scores; full T (dense tiles of later chunks read tokens 0:512
            # too)
            k8p_sb = ap.tile([128, 4, 2, T], dt8)
            # v natural + ones col per head (fp16 master copy for chunk 0's
            # fp16 diagonal PVs — only k-tiles 0..3 are read there; later
            # tiles stage through a rotating scratch before the fp8 split)
            v_sb = ap.tile([128, 4, 8 * 65], dts)
            # error-compensated fp8 (hi, err) pair for the DoubleRow PVs:
            # v ~= hi + err with both operands fp8e4m3 (the residual split
            # reconstructs v to ~0.1%). Interleaved [tt, 2, heads] so dense
            # tiles contract rows = (j0, j1) at fixed hi/err plane while
            # chunks 1-3's diag tiles contract rows = (hi, err) at fixed tt
            # against a broadcast exp. 68-wide head stride keeps every row
            # step 16B-aligned.
            v8p = ap.tile([128, TT, 2, 8 * 68], dt8)
            # normalized attention out (out-proj lhsT), one tile per token
            # chunk so out-proj of chunk c has no (tracker-level) dependency
            # on later chunks' y writes
            y_tiles = [ap.tile([128, 4, 512], dts, name=f"y{c}")
                       for c in range(TC)]

            # ones columns of v (col 64 of each head's block): 1.0 in the
            # master and the fp8-hi copy, 0.0 residual (1.0 is exact in fp8)
            v_heads = v_sb.rearrange("p t (h f) -> p t h f", h=8)
            v8p_heads = v8p.rearrange("p t r (h f) -> p t r h f", h=8)
            nc.vector.memset(v_heads[:, :, :, 64], 1.0)
            nc.vector.memset(v8p_heads[:, :, 0, :, 64], 1.0)
            nc.vector.memset(v8p_heads[:, :, 1, :, 64], 0.0)

            # ---- fine-grained work emitters -----------------------------
            def v_tile(tt):
                psv = pp.tile([128, 512], dt.float32, tag="ps512")
                for ks in range(KSUB):
                    nc.tensor.matmul(
                        psv[:],
                        xT_sb[:, tt // 4, ks, (tt % 4) * 128:(tt % 4 + 1) * 128],
                        Wv_sb[:, ks, :],
                        start=(ks == 0), stop=(ks == KSUB - 1),
                    )
                if tt < 4:
                    vdst = v_heads[:, tt, :, 0:64]
                else:
                    vtmp = sp.tile([128, 512], dts, tag="vtmp")
                    vdst = vtmp[:].rearrange("p (h f) -> p h f", h=8)
                nc.vector.tensor_add(
                    vdst,
                    psv[:].rearrange("p (h f) -> p h f", h=8),
                    bv_sb[:].rearrange("p (h f) -> p h f", h=8),
                )
                # fp8 hi + residual split on Pool (SBUF-only ops; keeps DVE
                # free for the PSUM-coupled work Pool cannot touch)
                nc.gpsimd.tensor_copy(
                    v8p_heads[:, tt, 0, :, 0:64], vdst)
                nc.gpsimd.tensor_sub(
                    v8p_heads[:, tt, 1, :, 0:64],
                    vdst, v8p_heads[:, tt, 0, :, 0:64])

            def proj_rope(fs, c4, use_big=False):
                tsl = slice(c4 * 512, (c4 + 1) * 512)
                if use_big:
                    # bootstrap: attention pools are idle, borrow a big tile
                    bigt = pb.tile([128, 1024], dt.float32, tag="big")
                    ps, rps = bigt[:, 0:512], bigt[:, 512:1024]
                else:
                    # single tile: the rope matmul reuses ps once the ACT
                    # bias-extract has read it, so two projs pipeline through
                    # the two pp buffers instead of one
                    ps = pp.tile([128, 512], dt.float32, tag="ps512")
                    rps = ps
                for ks in range(KSUB):
                    nc.tensor.matmul(
                        ps[:],
                        Wqk_sb[:, fs, ks, :],
                        xT_sb[:, c4, ks, :],
                        start=(ks == 0), stop=(ks == KSUB - 1),
                    )
                # PSUM evacuate + bias add; fp16 out makes the cos/sin muls
                # 2x-rate DVE ops. Early chunks' projections run while ACT
                # is exp-idle, so their evacuation goes to ACT; later
                # chunks' projections land inside exp-bound windows and use
                # DVE instead.
                qb = sp.tile([128, 512], dts, tag="qb")
                if c4 <= 1:
                    nc.scalar.activation(
                        qb[:], ps[:], mybir.ActivationFunctionType.Identity,
                        bias=bqk_sb[:, fs:fs + 1], scale=1.0)
                else:
                    nc.vector.tensor_scalar_add(
                        qb[:], ps[:], bqk_sb[:, fs:fs + 1])
                uw = sp.tile([128, 2, 512], dts, tag="uw")
                nc.vector.tensor_mul(
                    uw[:], qb[:, None, :].to_broadcast((128, 2, 512)),
                    sc_sb[:, :, tsl])
                nc.tensor.matmul(rps[:], RT_sb[:], uw[:, 0, :],
                                 start=True, stop=True)
                w = uw[:, 1, :]
                if fs < 4:
                    if c4 == 0:
                        nc.vector.tensor_add(qT_sb[:, fs, :], w, rps[:])
                    else:
                        nc.vector.tensor_add(q8_sb[:, fs, tsl], w, rps[:])
                else:
                    f4 = fs - 4
                    if c4 == 0:
                        nc.vector.tensor_add(kT_sb[:, f4, :], w, rps[:])
                        src = kT_sb[:, f4, :]
                    else:
                        kk = sp.tile([128, 512], dts, tag="kk")
                        nc.vector.tensor_add(kk[:], w, rps[:])
                        src = kk[:]
                    # (hi, err) fp8 split on the Pool engine (idle capacity)
                    nc.gpsimd.tensor_copy(k8p_sb[:, f4, 0, tsl], src)
                    nc.gpsimd.tensor_sub(
                        k8p_sb[:, f4, 1, tsl], src, k8p_sb[:, f4, 0, tsl])

            def out_proj(tt, n2):
                yt = y_tiles[tt // 4]
                t0 = (tt % 4) * 128
                po = pp.tile([128, 512], dt.float32, tag="ps512")
                for s in range(4):
                    nc.tensor.matmul(
                        po[:],
                        yt[:, s, t0:t0 + 128],
                        Wo_sb[:, s, n2 * 512:(n2 + 1) * 512],
                        start=(s == 0), stop=(s == 3),
                    )
                ost = op.tile([128, 512], dts, tag="ost")
                nc.vector.tensor_copy(ost[:], po[:])
                nc.sync.dma_start(
                    out[tt * 128:(tt + 1) * 128, n2 * 512:(n2 + 1) * 512],
                    ost[:],
                )

            from collections import deque

            # work items streamed into the attention pipeline's PE gaps.
            # Each item carries a due key (chunk, hp-block): block h of
            # chunk c reads only proj(h, c)/proj(h+4, c) plus the chunk's
            # v tiles (due before block 0), so later blocks' projections
            # overlap earlier blocks' exp-bound windows. out-proj items
            # (no due) are appended once a chunk's y is final and held
            # back preferentially for the (long) last chunk's j-loop.
            # dues are one block EARLY: the proj chain (PE -> bias -> rope
            # -> combine) takes ~2us to deliver, so just-in-time emission
            # stalls the consuming block's first scores
            fillers = deque()
            for fs in (1, 5, 2, 6, 3, 7):
                fillers.append(("proj", fs, 0, (0, max(0, fs % 4 - 1))))
            for c4 in range(1, TC):
                for tt in range(4 * c4, 4 * c4 + 4):
                    fillers.append(("v", tt, (c4 - 1, 3)))
                fillers.append(("proj", 0, c4, (c4 - 1, 3)))
                fillers.append(("proj", 4, c4, (c4 - 1, 3)))
                for fs in (1, 5, 2, 6, 3, 7):
                    fillers.append(("proj", fs, c4, (c4, fs % 4 - 1)))

            def out_epilogue():
                # last chunk's out-proj. All four token-tiles' s=0..2
                # accumulations (which depend only on the earlier blocks'
                # y writes) are emitted BEFORE any s=3 matmul, spread
                # across pb+pp+ppv PSUM so no hoist sits head-of-line
                # behind another tile's final-normalize wait; then the
                # s=3 tails, each followed by its copy (alternating
                # ACT/DVE) and store.
                yt = y_tiles[TC - 1]
                for tt in range(4 * TC - 4, 4 * TC):
                    t0 = (tt % 4) * 128
                    po2 = pb.tile([128, 1024], dt.float32, tag="big")
                    for n2 in range(2):
                        for s in range(3):
                            nc.tensor.matmul(
                                po2[:, n2 * 512:(n2 + 1) * 512],
                                yt[:, s, t0:t0 + 128],
                                Wo_sb[:, s, n2 * 512:(n2 + 1) * 512],
                                start=(s == 0), stop=False,
                                skip_group_check=True,
                            )
                    ost2 = op.tile([128, 1024], dts, tag="ost2")
                    for n2 in range(2):
                        nc.tensor.matmul(
                            po2[:, n2 * 512:(n2 + 1) * 512],
                            yt[:, 3, t0:t0 + 128],
                            Wo_sb[:, 3, n2 * 512:(n2 + 1) * 512],
                            start=False, stop=True,
                            skip_group_check=True,
                        )
                        if (tt + n2) % 2:
                            nc.scalar.activation(
                                ost2[:, n2 * 512:(n2 + 1) * 512],
                                po2[:, n2 * 512:(n2 + 1) * 512],
                                mybir.ActivationFunctionType.Identity,
                                bias=0.0, scale=1.0)
                        else:
                            nc.vector.tensor_copy(
                                ost2[:, n2 * 512:(n2 + 1) * 512],
                                po2[:, n2 * 512:(n2 + 1) * 512])
                        deng = nc.scalar if (tt + n2) % 2 else nc.sync
                        deng.dma_start(
                            out[tt * 128:(tt + 1) * 128,
                                n2 * 512:(n2 + 1) * 512],
                            ost2[:, n2 * 512:(n2 + 1) * 512])

            def run_item(it):
                if it[0] == "v":
                    v_tile(it[1])
                elif it[0] == "proj":
                    proj_rope(it[1], it[2])
                else:
                    out_proj(it[1], it[2])

            def drain_due(qc, hp):
                rest = deque()
                while fillers:
                    it = fillers.popleft()
                    if it[-1] is not None and it[-1] <= (qc, hp):
                        run_item(it)
                    else:
                        rest.append(it)
                fillers.extend(rest)

            # bootstrap: block 0's q/k + the chunk-0 v tiles; remaining
            # chunk-0 projections stream in as per-block due items
            proj_rope(0, 0, use_big=True)
            proj_rope(4, 0, use_big=True)
            for tt in range(4):
                v_tile(tt)

            # per-chunk filler quotas ~ the window's exp-vs-PE slack
            quotas = {0: 1, 1: 8, 2: 10}
            # diagonal (thin) tiles first (their longer chains overlap the
            # dense work). Chunk 0: fp16 scores; chunks 1-3: fp8 DoubleRow
            # at half cycles-per-column with k compensated (hi, err) rows
            # and q broadcast.
            def scores_j(qc, hp, j, qs, diag=False):
                big = pb.tile([128, 1024], dt.float32, tag="big")
                for par in range(2):
                    kb = par * 64
                    o = par * 512
                    if qc == 0:
                        nc.tensor.matmul(
                            big[:, o + qs:o + 512],
                            kT_sb[kb:kb + 64, hp, j * 128:(j + 1) * 128],
                            qT_sb[kb:kb + 64, hp, qs:512],
                            start=True, stop=not diag,
                            skip_group_check=diag,
                        )
                    else:
                        rhs = q8_sb[kb:kb + 64, hp,
                                    qc * 512 + qs:(qc + 1) * 512]
                        rhs = rhs[:, None, :].to_broadcast(
                            (64, 2, 512 - qs))
                        nc.tensor.matmul(
                            big[:, o + qs:o + 512],
                            k8p_sb[kb:kb + 64, hp, :,
                                   j * 128:(j + 1) * 128],
                            rhs,
                            start=True, stop=not diag,
                            perf_mode=mybir.MatmulPerfMode.DoubleRow,
                            skip_group_check=diag,
                        )
                    if diag:
                        # add -800 above the diagonal of the 128-col
                        # block at the diagonal; exp() then writes
                        # exact zeros there (no DVE op on the chain)
                        nc.tensor.matmul(
                            big[:, o + qs:o + qs + 128],
                            ident_sb[:],
                            mask_sb[:],
                            start=False, stop=True,
                            skip_group_check=True,
                        )
                return big.rearrange("p (two q) -> p two q", two=2)

            def build_parts(qc, hp):
                parts = [("s", 4 * qc + d) for d in range(4)]
                parts += [("d", 2 * m) for m in range(2 * qc)]
                if qc == TC - 1 and hp == 3:
                    # final block: keep one small diag tile (N=128 exp)
                    # last so the epilogue's normalize wait starts from
                    # a short exp, not a dense one
                    parts = parts[:3] + parts[4:] + [parts[3]]
                return parts

            def emit_part(qc, hp, pv0, pv1, pi, n_parts, kind, j0):
                last_part = pi == n_parts - 1
                if kind == "s":
                    qs = j0 * 128 - qc * 512
                    big_v = scores_j(qc, hp, j0, qs, diag=True)
                    # chunk 0 keeps fp16 exp + fp16 PV (early-token
                    # softmax rows are concentrated); chunks 1-3 go
                    # fp8 exp + (hi, err) DoubleRow PV at half rate
                    ex = ep.tile([128, 1024], dts if qc == 0 else dt8,
                                 tag="ex")
                    ex_v = ex.rearrange("p (two q) -> p two q", two=2)
                    nc.scalar.activation(
                        ex_v[:, :, qs:512], big_v[:, :, qs:512],
                        mybir.ActivationFunctionType.Exp,
                        bias=nb2[:], scale=0.125,
                    )
                    for par in range(2):
                        h = 2 * hp + par
                        pv = pv0 if par == 0 else pv1
                        if qc == 0:
                            nc.tensor.matmul(
                                pv[:, qs:512],
                                v_sb[:, j0, h * 65:(h + 1) * 65],
                                ex[:, par * 512 + qs:par * 512 + 512],
                                start=(pi == 0), stop=last_part,
                                skip_group_check=True,
                            )
                        else:
                            exb = ex[:, None,
                                     par * 512 + qs:par * 512 + 512]
                            nc.tensor.matmul(
                                pv[:, qs:512],
                                v8p[:, j0, :,
                                    h * 68:h * 68 + 65],
                                exb.to_broadcast((128, 2, 512 - qs)),
                                start=(pi == 0), stop=last_part,
                                skip_group_check=True,
                                perf_mode=mybir.MatmulPerfMode.DoubleRow,
                            )
                else:
                    ex2 = ep.tile([128, 2, 1024], dt8, tag="ex2")
                    for i in range(2):
                        big_v = scores_j(qc, hp, j0 + i, 0)
                        nc.scalar.activation(
                            ex2[:, i].rearrange(
                                "p (two q) -> p two q", two=2)[:, :, :],
                            big_v[:, :, :],
                            mybir.ActivationFunctionType.Exp,
                            bias=nb2[:], scale=0.125,
                        )
                    for par in range(2):
                        h = 2 * hp + par
                        pv = pv0 if par == 0 else pv1
                        for rr in range(2):
                            nc.tensor.matmul(
                                pv[:, 0:512],
                                v8p[:, j0:j0 + 2, rr,
                                    h * 68:h * 68 + 65],
                                ex2[:, :, par * 512:par * 512 + 512],
                                start=False,
                                stop=(last_part and rr == 1),
                                skip_group_check=True,
                                perf_mode=mybir.MatmulPerfMode.DoubleRow,
                            )

            peeled = None  # (pv0, pv1) with the next block's part 0 emitted
            for qc in range(TC):
                iters = 4 * (4 * qc + 4)
                # all out-proj work is held for the last (exp-bound)
                # chunk's j-loop; the dedicated out_epilogue covers the
                # final-normalize tail
                if qc == TC - 1:
                    quota = max(0, len(fillers) - 12)
                else:
                    quota = quotas[qc]
                pops = it_count = 0
                for hp in range(4):
                    drain_due(qc, hp)
                    parts = build_parts(qc, hp)
                    if peeled is not None:
                        pv0, pv1 = peeled
                        peeled = None
                        start_pi = 1
                        it_count += 1
                    else:
                        pv0 = ppv.tile([65, 512], dt.float32, tag="pv")
                        pv1 = ppv.tile([65, 512], dt.float32, tag="pv")
                        start_pi = 0
                    for pi in range(start_pi, len(parts)):
                        kind, j0 = parts[pi]
                        emit_part(qc, hp, pv0, pv1, pi, len(parts), kind, j0)
                        it_count += 2 if kind == "d" else 1
                        while (pi != len(parts) - 1 and fillers
                               and pops < quota
                               and pops * iters < quota * it_count):
                            run_item(fillers.popleft())
                            pops += 1
                    last = qc == TC - 1 and hp == 3
                    pvs = []
                    for par in range(2):
                        pv = pv0 if par == 0 else pv1
                        if not last:
                            # free the pv PSUM bank fast (the next block's
                            # first PV reuses it): evacuate to SBUF, then
                            # normalize from the copy. Without this the
                            # recip->broadcast->mul chain holds the bank
                            # ~2us and stalls the next block's PV start.
                            pvc = np_.tile([65, 512], dt.float32, tag="pvc")
                            nc.vector.tensor_copy(pvc[:], pv[:])
                            pv = pvc
                        pvs.append(pv)
                    # (a cross-chunk peel of the next chunk's first part
                    # was tried here and measured slower: the boundary
                    # exps queue behind the saturated ACT engine anyway)
                    for par in range(2):
                        kb = par * 64
                        pv = pvs[par]
                        rinv = np_.tile([1, 512], dt.float32, tag="rinv")
                        nc.vector.reciprocal(rinv[0:1, :], pv[64:65, :])
                        rb = np_.tile([64, 512], dt.float32, tag="rb")
                        nc.gpsimd.partition_broadcast(rb[:], rinv[0:1, :])
                        nc.vector.tensor_mul(
                            y_tiles[qc][kb:kb + 64, hp, :], pv[0:64, :], rb[:],
                        )
                    while (fillers and pops < quota
                           and pops * iters < quota * it_count):
                        run_item(fillers.popleft())
                        pops += 1
                    if hp == 3 and qc < TC - 1:
                        # this chunk's y is final for all heads: stream
                        # out-proj (half-width items for finer PE-gap
                        # packing)
                        for tt in range(4 * qc, 4 * qc + 4):
                            fillers.append(("out", tt, 0, None))
                            fillers.append(("out", tt, 1, None))
            while fillers:
                run_item(fillers.popleft())
            out_epilogue()

    nc.compile()
    return nc


def _prep_core_inputs(x, W_qkv, b_qkv, W_out, g):
    """Host-side shard prep for head-group g (features g*512:(g+1)*512)."""
    fs = slice(g * 512, (g + 1) * 512)
    Wq = W_qkv[:, 0:1024][:, fs]          # [1024, 512]
    Wk = W_qkv[:, 1024:2048][:, fs]
    Wv_ = W_qkv[:, 2048:3072][:, fs]
    bq = b_qkv[0:1024][fs]
    bk = b_qkv[1024:2048][fs]
    bv_ = b_qkv[2048:3072][fs]

    Wqk_np = np.concatenate([Wq, Wk], axis=1)        # [1024, 1024]
    # [fs, p, ks, col]
    Wqk_np = Wqk_np.reshape(KSUB, 128, 8, 128).transpose(2, 1, 0, 3)
    Wv_np = Wv_.reshape(KSUB, 128, 512).transpose(1, 0, 2)
    Wo_np = W_out[fs, :].reshape(4, 128, 1024).transpose(1, 0, 2)
    bqk_np = np.concatenate([bq, bk]).reshape(8, 128).T.copy()   # [128, 8]
    bv_np = np.broadcast_to(bv_[None, :], (128, 512)).copy()

    return {
        "Wqk": np.ascontiguousarray(Wqk_np).astype(f16),
        "Wv": np.ascontiguousarray(Wv_np).astype(f16),
        "Wo": np.ascontiguousarray(Wo_np).astype(f16),
        "bqk": np.ascontiguousarray(bqk_np).astype(np.float32),
        "bv": bv_np.astype(np.float32),
    }


def _shared_inputs():
    # rotation matrix: (R q)[d] = -q[d+32] for d<32, q[d-32] for 32<=d<64
    R64 = np.zeros((64, 64), dtype=np.float32)
    for d in range(32):
        R64[d, d + 32] = -1.0
        R64[d + 32, d] = 1.0
    R128 = np.zeros((128, 128), dtype=np.float32)
    R128[0:64, 0:64] = R64
    R128[64:128, 64:128] = R64
    RT_np = R128.T.copy()

    inv_freq = 1.0 / (10000.0 ** (np.arange(0, HD, 2, dtype=np.float32) / HD))
    t = np.arange(T, dtype=np.float32)
    freqs = np.outer(t, inv_freq)                     # [T, 32]
    p = np.arange(128)
    cos_np = np.cos(freqs[:, p % 32]).T.copy()        # [128, T]
    sin_np = np.sin(freqs[:, p % 32]).T.copy()

    # additive causal mask for the diagonal 128-block, [kj, q] orientation:
    # -800 where kj > q (exp maps it to an exact 0), applied to the scores
    # PSUM via a matmul against the identity
    mask_np = -800.0 * np.tril(np.ones((128, 128), dtype=np.float32), -1)
    ident_np = np.eye(128, dtype=np.float32)

    return {
        "RT": RT_np.astype(f16),
        "cosd": np.ascontiguousarray(
            cos_np.reshape(128, TC, 512).transpose(1, 0, 2)).astype(f16),
        "sind": np.ascontiguousarray(
            sin_np.reshape(128, TC, 512).transpose(1, 0, 2)).astype(f16),
        "maskd": np.ascontiguousarray(mask_np).astype(f16),
        "identd": ident_np.astype(f16),
    }


def run(x, W_qkv, b_qkv, W_out, b_out, trace=False):
    global _compiled
    if _compiled is None:
        _compiled = _build()
    nc = _compiled

    shared = _shared_inputs()
    group_inp = [_prep_core_inputs(x, W_qkv, b_qkv, W_out, g) for g in range(2)]

    in_maps = []
    for core in range(N_CORES):
        b, g = core // 2, core % 2
        # [c4, p, ks, q]
        xT_np = (x[b].reshape(TC, 512, KSUB, 128).transpose(0, 3, 2, 1))
        m = {"xT": np.ascontiguousarray(xT_np).astype(f16)}
        m.update(group_inp[g])
        m.update(shared)
        in_maps.append(m)

    res = run_bass_kernel_spmd(
        nc, in_maps, core_ids=list(range(N_CORES)), trace=trace,
        stitch_traces=trace,
    )
    outp = np.empty((B, T, C), dtype=np.float32)
    for b in range(B):
        outp[b] = (res.results[2 * b]["out"].astype(np.float32)
                   + res.results[2 * b + 1]["out"].astype(np.float32)
                   + b_out[None, :])
    return outp, res


def kernel(x, W_qkv, b_qkv, W_out, b_out):
    x = np.asarray(x, dtype=np.float32)
    W_qkv = np.asarray(W_qkv, dtype=np.float32)
    b_qkv = np.asarray(b_qkv, dtype=np.float32)
    W_out = np.asarray(W_out, dtype=np.float32)
    b_out = np.asarray(b_out, dtype=np.float32)
    outp, _ = run(x, W_qkv, b_qkv, W_out, b_out, trace=False)
    return outp
